# revision 5
# baseline (speedup 1.0000x reference)
"""Trainium2 Bass kernel for nn_GATSampling (2-layer bipartite GAT, 8 NeuronCores).

Strategy (SPMD over 8 cores, host re-shards between launches):

  Slot-major edge layout: destination nodes are ranked by degree and dealt
  into blocks of 128 consecutive ranks; global block gb -> core gb%8,
  program slot gb//8.  Partition lane p holds the edges of slot p, padded
  to a per-GROUP uniform K (multiple of 4).  The segment-sum is a PSUM
  accumulation of 512-col matmuls against a constant identity stationary
  (4 k-chunks per matmul, last dim packed); a DVE tensor_reduce folds the
  quad lanes.

  Launch T: feature transform with column-stationary float32r matmuls
            (psum = W^T @ chunk^T, 512 cols per matmul); el/er attention
            terms are transposed on the PE so every output DMA uses all
            128 partitions.
  Host:     gather per-edge rows into per-core streams laid out
            [lane, j, h, d, k] (k innermost: DVE 2x multiply).
  Launch A: layer-0 edge phase, group-fused: e = el+er, s = exp(prelu(e)),
            a = s/sum_k s, m = fs*a are single instructions per 8-block
            group; per block quad matmuls + fold; ELU via Act relu/exp
            (pair-batched, -1 folded into the transpose copy's bias);
            h1ext = elu @ [W1 | W1@al1m | W1@ar1m] in bf16.
  Launch B: layer-2 edge phase; quad-fold and head-mean fused into one
            strided tensor_reduce; outputs logits.
"""
import sys

sys.path.insert(0, "/opt/trn_rl_repo")

import numpy as np
import ml_dtypes

from concourse import bass, mybir, tile, bacc, bass_utils

F32 = mybir.dt.float32
F32R = mybir.dt.float32r
BF16 = mybir.dt.bfloat16
NPBF = ml_dtypes.bfloat16
P = 128
NCORES = 8
NEG_SLOPE = 0.2
H, D = 4, 32
HD = H * D  # 128
PARW = 4    # k-chunks folded into one 512-col matmul

N0, N1, N2 = 200000, 50000, 12500
E0, E1 = 800000, 200000
F_IN = 128

T0_CHUNKS = 196
T0_ROWS = T0_CHUNKS * P         # 25088
T1_CHUNKS = 49
T1_ROWS = T1_CHUNKS * P         # 6272

NBLK0 = 50                      # layer-0 dst blocks per core (even, >= 49)
NBLK1 = 14                      # layer-2 dst blocks per core (even, >= 13)
GROUPS0 = [(0, 2)] + [(2 + 8 * i, 10 + 8 * i) for i in range(6)]
GROUPS1 = [(0, 2), (2, 6), (6, 10), (10, 14)]

EL_PAD = -87.0

_cache = {}


# --------------------------------------------------------------------------
# host-side graph partitioning (index bookkeeping only)
# --------------------------------------------------------------------------
def _partition(dst, n_dst, nblk_core, groups):
    deg = np.bincount(dst, minlength=n_dst)
    order = np.argsort(-deg, kind="stable")
    rank = np.empty(n_dst, np.int64)
    rank[order] = np.arange(n_dst)
    degs = deg[order]
    Khat = np.empty(nblk_core, np.int64)
    for j in range(nblk_core):
        lo = (j * NCORES) * P
        Khat[j] = degs[lo] if lo < n_dst else 1
    # uniform K per group, multiple of PARW
    for (j0, j1) in groups:
        Kg = int(-(-Khat[j0:j1].max() // PARW) * PARW)
        Khat[j0:j1] = Kg
    start = np.zeros(nblk_core + 1, np.int64)
    np.cumsum(Khat, out=start[1:])
    return order, rank, Khat, start


def _edge_place(dst, rank, start):
    r = rank[dst]
    eorder = np.argsort(r, kind="stable")
    rs = r[eorder]
    first = np.searchsorted(rs, rs)
    k = np.arange(len(rs)) - first
    gb = rs >> 7
    core = gb & (NCORES - 1)
    j = gb >> 3
    lane = rs & (P - 1)
    chunk = start[j] + k
    return eorder, core, chunk, lane


def _build_streams(fs_rows_u16, el_rows, src, eorder, core, chunk, lane,
                   Khat, start, nblk_core):
    C = int(start[-1])
    arr_fs = np.zeros((NCORES, C, P, HD), np.uint16)
    arr_el = np.full((NCORES, C, P, H), EL_PAD, np.float32)
    se = src[eorder]
    arr_fs[core, chunk, lane] = fs_rows_u16[se]
    arr_el[core, chunk, lane] = el_rows[se]
    sfs = np.empty((NCORES, P, HD * C), np.uint16)
    sel = np.empty((NCORES, P, H * C), np.float32)
    for j in range(nblk_core):
        s0, K = int(start[j]), int(Khat[j])
        fslab = arr_fs[:, s0:s0 + K].transpose(0, 2, 3, 1)
        sfs[:, :, HD * s0:HD * (s0 + K)] = fslab.reshape(NCORES, P, HD * K)
        eslab = arr_el[:, s0:s0 + K].transpose(0, 2, 3, 1)
        sel[:, :, H * s0:H * (s0 + K)] = eslab.reshape(NCORES, P, H * K)
    return sfs, sel


def _per_slot_table(vals_by_rank, nblk_core):
    nb = nblk_core * NCORES
    v = np.zeros((nb * P, H), np.float32)
    v[:len(vals_by_rank)] = vals_by_rank
    v = v.reshape(nblk_core, NCORES, P, H)
    return np.ascontiguousarray(v.transpose(1, 2, 0, 3)).reshape(
        NCORES, P, nblk_core * H)


# --------------------------------------------------------------------------
# bass programs
# --------------------------------------------------------------------------
def _build_T():
    GC = 4          # chunks per matmul batch (512 cols)
    ST = 28         # chunks per dma stage
    nc = bacc.Bacc("TRN2", target_bir_lowering=False, debug=False)
    f0T = nc.dram_tensor("f0T", [P, T0_CHUNKS * P], F32R, kind="ExternalInput").ap()
    f1T = nc.dram_tensor("f1T", [P, T1_CHUNKS * P], F32R, kind="ExternalInput").ap()
    w0 = nc.dram_tensor("w0", [F_IN, HD], F32R, kind="ExternalInput").ap()
    w0al = nc.dram_tensor("w0al", [F_IN, H], F32R, kind="ExternalInput").ap()
    w0ar = nc.dram_tensor("w0ar", [F_IN, H], F32R, kind="ExternalInput").ap()
    ident4 = nc.dram_tensor("ident4", [H, H], F32, kind="ExternalInput").ap()
    fs0bT = nc.dram_tensor("fs0bT", [P, T0_ROWS], BF16, kind="ExternalOutput").ap()
    el0i = nc.dram_tensor("el0i", [P, T0_CHUNKS * H], F32, kind="ExternalOutput").ap()
    er0i = nc.dram_tensor("er0i", [P, T1_CHUNKS * H], F32, kind="ExternalOutput").ap()

    with tile.TileContext(nc) as tc:
        with (
            tc.tile_pool(name="const", bufs=1) as cpool,
            tc.tile_pool(name="load", bufs=2) as lpool,
            tc.tile_pool(name="fsout", bufs=2) as fpool,
            tc.tile_pool(name="elout", bufs=2) as epool,
            tc.tile_pool(name="work", bufs=2) as wpool,
            tc.tile_pool(name="ps", bufs=2, space="PSUM") as ppool,
            tc.tile_pool(name="pse", bufs=2, space="PSUM") as ppool2,
            tc.tile_pool(name="pst", bufs=2, space="PSUM") as ppool3,
        ):
            w0_sb = cpool.tile([F_IN, HD], F32R)
            nc.sync.dma_start(w0_sb[:], w0)
            w0al_sb = cpool.tile([F_IN, H], F32R)
            nc.sync.dma_start(w0al_sb[:], w0al)
            w0ar_sb = cpool.tile([F_IN, H], F32R)
            nc.sync.dma_start(w0ar_sb[:], w0ar)
            id4_sb = cpool.tile([H, H], F32)
            nc.sync.dma_start(id4_sb[:], ident4)

            def elpass(ld, g, w_sb, eout, base_chunk):
                """el/er transform for 4 chunks + transpose to lane-major."""
                sl = slice(g * GC * P, (g + 1) * GC * P)
                pse = ppool2.tile([H, GC * P], F32, space="PSUM", tag="pse")
                nc.tensor.matmul(pse[:], lhsT=w_sb[:], rhs=ld[:, sl],
                                 start=True, stop=True)
                esb = wpool.tile([H, GC * P], F32, tag="esb")
                nc.vector.tensor_scalar(out=esb[:], in0=pse[:], scalar1=1.0,
                                        scalar2=None, op0=mybir.AluOpType.mult)
                pt = ppool3.tile([P, GC * H], F32, space="PSUM", tag="pt")
                for c in range(GC):
                    nc.tensor.transpose(
                        out=pt[:, c * H:(c + 1) * H],
                        in_=esb[:, c * P:(c + 1) * P], identity=id4_sb[:])
                nc.scalar.copy(
                    eout[:, (base_chunk + g * GC) * H:
                         (base_chunk + (g + 1) * GC) * H], pt[:])

            # ---- feat0 pass ----
            el_st = cpool.tile([P, T0_CHUNKS * H], F32)
            for st in range(T0_CHUNKS // ST):
                ld = lpool.tile([P, ST * P], F32R, tag="ld")
                nc.sync.dma_start(ld[:], f0T[:, st * ST * P:(st + 1) * ST * P])
                fst = fpool.tile([P, ST * P], BF16, tag="fst")
                for g in range(ST // GC):
                    sl = slice(g * GC * P, (g + 1) * GC * P)
                    ps = ppool.tile([P, GC * P], F32, space="PSUM", tag="ps")
                    nc.tensor.matmul(ps[:], lhsT=w0_sb[:], rhs=ld[:, sl],
                                     start=True, stop=True)
                    nc.scalar.copy(fst[:, sl], ps[:])
                    elpass(ld, g, w0al_sb, el_st, st * ST)
                nc.sync.dma_start(
                    fs0bT[:, st * ST * P:(st + 1) * ST * P], fst[:])
            nc.sync.dma_start(el0i, el_st[:])

            # ---- feat1 pass ----
            er_st = cpool.tile([P, T1_CHUNKS * H], F32)
            for st in range(2):
                c0 = st * ST
                c1 = min(T1_CHUNKS, (st + 1) * ST)
                ld = lpool.tile([P, ST * P], F32R, tag="ld")
                nc.sync.dma_start(ld[:, 0:(c1 - c0) * P],
                                  f1T[:, c0 * P:c1 * P])
                for g in range((c1 - c0) // GC):
                    elpass(ld, g, w0ar_sb, er_st, c0)
                # remainder chunk (49 = 12*4 + 1)
                r0 = c0 + ((c1 - c0) // GC) * GC
                if r0 < c1:
                    pse = ppool2.tile([H, GC * P], F32, space="PSUM", tag="pse")
                    nc.tensor.matmul(
                        pse[:, 0:(c1 - r0) * P], lhsT=w0ar_sb[:],
                        rhs=ld[:, (r0 - c0) * P:(c1 - c0) * P],
                        start=True, stop=True)
                    esb = wpool.tile([H, GC * P], F32, tag="esb")
                    nc.vector.tensor_scalar(
                        out=esb[:, 0:(c1 - r0) * P],
                        in0=pse[:, 0:(c1 - r0) * P], scalar1=1.0,
                        scalar2=None, op0=mybir.AluOpType.mult)
                    pt = ppool3.tile([P, GC * H], F32, space="PSUM", tag="pt")
                    for c in range(c1 - r0):
                        nc.tensor.transpose(
                            out=pt[:, c * H:(c + 1) * H],
                            in_=esb[:, c * P:(c + 1) * P], identity=id4_sb[:])
                    nc.scalar.copy(
                        er_st[:, r0 * H:c1 * H], pt[:, 0:(c1 - r0) * H])
            nc.sync.dma_start(er0i, er_st[:])

    nc.compile()
    return nc


def _group_attention(nc, Gv5, Ev, er_b, wpool, J, K, recip_scale):
    """Group-fused attention: in-place m = fs*a on Gv5 [P,J,H,D,K]."""
    et = wpool.tile([P, J, H, K], F32, tag="et")
    nc.vector.tensor_tensor(out=et[:], in0=Ev, in1=er_b,
                            op=mybir.AluOpType.add)
    lr = wpool.tile([P, J, H, K], F32, tag="lr")
    nc.scalar.activation(out=lr[:], in_=et[:],
                         func=mybir.ActivationFunctionType.Prelu,
                         alpha=NEG_SLOPE)
    s = wpool.tile([P, J, H, K], BF16, tag="s")
    nc.scalar.activation(out=s[:], in_=lr[:],
                         func=mybir.ActivationFunctionType.Exp)
    ssum = wpool.tile([P, J, H], F32, tag="ssum")
    nc.vector.tensor_reduce(out=ssum[:], in_=s[:],
                            axis=mybir.AxisListType.X,
                            op=mybir.AluOpType.add)
    if recip_scale != 1.0:
        nc.vector.tensor_scalar(out=ssum[:], in0=ssum[:],
                                scalar1=recip_scale, scalar2=None,
                                op0=mybir.AluOpType.mult)
    rec = wpool.tile([P, J, H], F32, tag="rec")
    nc.vector.reciprocal(rec[:], ssum[:])
    a = wpool.tile([P, J, H, K], BF16, tag="a")
    nc.vector.tensor_tensor(
        out=a[:], in0=s[:],
        in1=rec[:].unsqueeze(3).to_broadcast([P, J, H, K]),
        op=mybir.AluOpType.mult)
    nc.vector.tensor_tensor(
        out=Gv5, in0=Gv5,
        in1=a[:].unsqueeze(3).to_broadcast([P, J, H, D, K]),
        op=mybir.AluOpType.mult)


def _seg_matmuls(nc, psq, Gvj, identb_sb, K):
    nq = K // PARW
    Gq = Gvj.rearrange("p h d (kk par) -> p h d kk par", par=PARW)
    for kk in range(nq):
        nc.tensor.matmul(psq[:], lhsT=identb_sb[:],
                         rhs=Gq[:, :, :, kk, :],
                         start=(kk == 0), stop=(kk == nq - 1))


def _build_A(Khat):
    nblk = len(Khat)
    start = np.zeros(nblk + 1, np.int64)
    np.cumsum(Khat, out=start[1:])
    C = int(start[-1])
    maxw = max(int(start[j1] - start[j0]) for j0, j1 in GROUPS0)

    nc = bacc.Bacc("TRN2", target_bir_lowering=False, debug=False)
    sfs = nc.dram_tensor("sfs", [P, HD * C], BF16, kind="ExternalInput").ap()
    sel = nc.dram_tensor("sel", [P, H * C], F32, kind="ExternalInput").ap()
    ers = nc.dram_tensor("ers", [P, nblk * H], F32, kind="ExternalInput").ap()
    identb = nc.dram_tensor("identb", [P, P], BF16, kind="ExternalInput").ap()
    w1b = nc.dram_tensor("w1b", [HD, 136], BF16, kind="ExternalInput").ap()
    obfs = nc.dram_tensor("obfs", [P, nblk * HD], BF16, kind="ExternalOutput").ap()
    obf8 = nc.dram_tensor("obf8", [P, nblk * 8], F32, kind="ExternalOutput").ap()

    with tile.TileContext(nc) as tc:
        with (
            tc.tile_pool(name="const", bufs=1) as cpool,
            tc.tile_pool(name="gload", bufs=2) as gpool,
            tc.tile_pool(name="eload", bufs=2) as epool,
            tc.tile_pool(name="work", bufs=2) as wpool,
            tc.tile_pool(name="pwork", bufs=2) as qpool,
            tc.tile_pool(name="stage", bufs=1) as spool,
            tc.tile_pool(name="psm", bufs=2, space="PSUM") as ppool,
            tc.tile_pool(name="pst", bufs=2, space="PSUM") as ppool2,
            tc.tile_pool(name="ps3", bufs=2, space="PSUM") as ppool3,
        ):
            identb_sb = cpool.tile([P, P], BF16)
            nc.sync.dma_start(identb_sb[:], identb)
            w1b_sb = cpool.tile([HD, 136], BF16)
            nc.sync.dma_start(w1b_sb[:], w1b)
            ers_sb = cpool.tile([P, nblk * H], F32)
            nc.sync.dma_start(ers_sb[:], ers)
            ofs_st = spool.tile([P, nblk * HD], BF16)
            of8_st = spool.tile([P, nblk * 8], F32)

            for (j0, j1) in GROUPS0:
                J = j1 - j0
                K = int(Khat[j0])
                s0 = int(start[j0])
                w = J * K
                Gg = gpool.tile([P, HD * maxw], BF16, tag="G")
                nc.sync.dma_start(Gg[:, 0:HD * w],
                                  sfs[:, HD * s0:HD * (s0 + w)])
                Eg = epool.tile([P, H * maxw], F32, tag="E")
                nc.sync.dma_start(Eg[:, 0:H * w],
                                  sel[:, H * s0:H * (s0 + w)])
                Gv5 = Gg[:, 0:HD * w].rearrange(
                    "p (j h d k) -> p j h d k", j=J, h=H, d=D)
                Ev = Eg[:, 0:H * w].rearrange(
                    "p (j h k) -> p j h k", j=J, h=H)
                er_b = ers_sb[:, j0 * H:j1 * H].rearrange(
                    "p (j h) -> p j h", j=J).unsqueeze(3).to_broadcast(
                    [P, J, H, K])
                _group_attention(nc, Gv5, Ev, er_b, wpool, J, K, 1.0)
                for pp in range(J // 2):
                    yp = qpool.tile([P, 2, HD], F32, tag="yp")
                    ps3p = ppool3.tile([P, 2, 136], F32, space="PSUM",
                                       tag="ps3")
                    for jj in range(2):
                        psq = ppool.tile([P, HD, PARW], F32, space="PSUM",
                                         tag="psq")
                        _seg_matmuls(nc, psq, Gv5[:, pp * 2 + jj],
                                     identb_sb, K)
                        nc.vector.tensor_reduce(out=yp[:, jj, :], in_=psq[:],
                                                axis=mybir.AxisListType.X,
                                                op=mybir.AluOpType.add)
                    r1p = qpool.tile([P, 2, HD], BF16, tag="r1p")
                    nc.scalar.activation(out=r1p[:], in_=yp[:],
                                         func=mybir.ActivationFunctionType.Relu)
                    ngp = qpool.tile([P, 2, HD], F32, tag="ngp")
                    nc.scalar.activation(out=ngp[:], in_=yp[:],
                                         func=mybir.ActivationFunctionType.Relu,
                                         scale=-1.0)
                    exp2 = qpool.tile([P, 2, HD], BF16, tag="exp2")
                    nc.scalar.activation(out=exp2[:], in_=ngp[:],
                                         func=mybir.ActivationFunctionType.Exp,
                                         scale=-1.0)
                    elu1p = qpool.tile([P, 2, HD], BF16, tag="elu1p")
                    nc.vector.tensor_tensor(out=elu1p[:], in0=r1p[:],
                                            in1=exp2[:],
                                            op=mybir.AluOpType.add)
                    for jj in range(2):
                        pst = ppool2.tile([P, P], BF16, space="PSUM",
                                          tag="pst")
                        nc.tensor.transpose(out=pst[:], in_=elu1p[:, jj, :],
                                            identity=identb_sb[:])
                        eluT = wpool.tile([P, P], BF16, tag="eluT")
                        nc.scalar.activation(
                            out=eluT[:], in_=pst[:],
                            func=mybir.ActivationFunctionType.Copy,
                            bias=-1.0)
                        nc.tensor.matmul(ps3p[:, jj, :], lhsT=eluT[:],
                                         rhs=w1b_sb[:], start=True, stop=True)
                    jb = j0 + pp * 2
                    nc.scalar.copy(ofs_st[:, jb * HD:(jb + 2) * HD],
                                   ps3p[:, :, 0:HD])
                    nc.scalar.copy(of8_st[:, jb * 8:(jb + 2) * 8],
                                   ps3p[:, :, HD:HD + 8])
            nc.sync.dma_start(obfs, ofs_st[:])
            nc.sync.dma_start(obf8, of8_st[:])

    nc.compile()
    return nc


def _build_B(Khat):
    nblk = len(Khat)
    start = np.zeros(nblk + 1, np.int64)
    np.cumsum(Khat, out=start[1:])
    C = int(start[-1])
    maxw = max(int(start[j1] - start[j0]) for j0, j1 in GROUPS1)

    nc = bacc.Bacc("TRN2", target_bir_lowering=False, debug=False)
    sfs = nc.dram_tensor("sfs", [P, HD * C], BF16, kind="ExternalInput").ap()
    sel = nc.dram_tensor("sel", [P, H * C], F32, kind="ExternalInput").ap()
    ers = nc.dram_tensor("ers", [P, nblk * H], F32, kind="ExternalInput").ap()
    identb = nc.dram_tensor("identb", [P, P], BF16, kind="ExternalInput").ap()
    olog = nc.dram_tensor("olog", [P, nblk * D], F32, kind="ExternalOutput").ap()

    with tile.TileContext(nc) as tc:
        with (
            tc.tile_pool(name="const", bufs=1) as cpool,
            tc.tile_pool(name="gload", bufs=2) as gpool,
            tc.tile_pool(name="eload", bufs=2) as epool,
            tc.tile_pool(name="work", bufs=2) as wpool,
            tc.tile_pool(name="stage", bufs=1) as spool,
            tc.tile_pool(name="psm", bufs=2, space="PSUM") as ppool,
        ):
            identb_sb = cpool.tile([P, P], BF16)
            nc.sync.dma_start(identb_sb[:], identb)
            ers_sb = cpool.tile([P, nblk * H], F32)
            nc.sync.dma_start(ers_sb[:], ers)
            olog_st = spool.tile([P, nblk * D], F32)

            for (j0, j1) in GROUPS1:
                J = j1 - j0
                K = int(Khat[j0])
                s0 = int(start[j0])
                w = J * K
                Gg = gpool.tile([P, HD * maxw], BF16, tag="G")
                nc.sync.dma_start(Gg[:, 0:HD * w],
                                  sfs[:, HD * s0:HD * (s0 + w)])
                Eg = epool.tile([P, H * maxw], F32, tag="E")
                nc.sync.dma_start(Eg[:, 0:H * w],
                                  sel[:, H * s0:H * (s0 + w)])
                Gv5 = Gg[:, 0:HD * w].rearrange(
                    "p (j h d k) -> p j h d k", j=J, h=H, d=D)
                Ev = Eg[:, 0:H * w].rearrange(
                    "p (j h k) -> p j h k", j=J, h=H)
                er_b = ers_sb[:, j0 * H:j1 * H].rearrange(
                    "p (j h) -> p j h", j=J).unsqueeze(3).to_broadcast(
                    [P, J, H, K])
                _group_attention(nc, Gv5, Ev, er_b, wpool, J, K, 4.0)
                for jj in range(J):
                    j = j0 + jj
                    psq = ppool.tile([P, HD, PARW], F32, space="PSUM",
                                     tag="psq")
                    _seg_matmuls(nc, psq, Gv5[:, jj], identb_sb, K)
                    nc.vector.tensor_reduce(
                        out=olog_st[:, j * D:(j + 1) * D],
                        in_=psq[:].rearrange("p (h d) par -> p d h par", h=H),
                        axis=mybir.AxisListType.XY,
                        op=mybir.AluOpType.add)
            nc.sync.dma_start(olog, olog_st[:])

    nc.compile()
    return nc


def _get_programs(Khat0, Khat1):
    key = (tuple(Khat0), tuple(Khat1))
    if key not in _cache:
        _cache[key] = (_build_T(), _build_A(Khat0), _build_B(Khat1))
    return _cache[key]


def _run(nc, in_maps, trace=False):
    return bass_utils.run_bass_kernel_spmd(
        nc, in_maps, list(range(NCORES)), trace=trace)


# --------------------------------------------------------------------------
# main entry
# --------------------------------------------------------------------------
def kernel(feat0, feat1, src0, dst0, src1, dst1, map12,
           W0, al0, ar0, W1, al1, ar1, _collect_times=None, _trace=False):
    feat0 = np.asarray(feat0, np.float32)
    feat1 = np.asarray(feat1, np.float32)
    src0 = np.asarray(src0).astype(np.int64)
    dst0 = np.asarray(dst0).astype(np.int64)
    src1 = np.asarray(src1).astype(np.int64)
    dst1 = np.asarray(dst1).astype(np.int64)
    map12 = np.asarray(map12).astype(np.int64)
    W0 = np.asarray(W0, np.float32)
    W1 = np.asarray(W1, np.float32)
    al0 = np.asarray(al0, np.float32); ar0 = np.asarray(ar0, np.float32)
    al1 = np.asarray(al1, np.float32); ar1 = np.asarray(ar1, np.float32)

    al0m = np.zeros((HD, H), np.float32)
    ar0m = np.zeros((HD, H), np.float32)
    al1m = np.zeros((HD, H), np.float32)
    ar1m = np.zeros((HD, H), np.float32)
    for h in range(H):
        al0m[h * D:(h + 1) * D, h] = al0[h]
        ar0m[h * D:(h + 1) * D, h] = ar0[h]
        al1m[h * D:(h + 1) * D, h] = al1[h]
        ar1m[h * D:(h + 1) * D, h] = ar1[h]
    W0al = (W0 @ al0m).astype(np.float32)
    W0ar = (W0 @ ar0m).astype(np.float32)
    W1full_b = np.concatenate(
        [W1, W1 @ al1m, W1 @ ar1m], axis=1).astype(NPBF)
    ident_b = np.eye(P, dtype=NPBF)
    ident4 = np.eye(H, dtype=np.float32)

    order0, rank0, Khat0, start0 = _partition(dst0, N1, NBLK0, GROUPS0)
    order1, rank1, Khat1, start1 = _partition(dst1, N2, NBLK1, GROUPS1)

    ncT, ncA, ncB = _get_programs(Khat0, Khat1)

    # ---- launch T ----
    f0pad = np.zeros((NCORES * T0_ROWS, F_IN), np.float32)
    f0pad[:N0] = feat0
    f1pad = np.zeros((NCORES * T1_ROWS, F_IN), np.float32)
    f1pad[:N1] = feat1
    t_maps = []
    for c in range(NCORES):
        t_maps.append({
            "f0T": np.ascontiguousarray(
                f0pad[c * T0_ROWS:(c + 1) * T0_ROWS].T),
            "f1T": np.ascontiguousarray(
                f1pad[c * T1_ROWS:(c + 1) * T1_ROWS].T),
            "w0": W0, "w0al": W0al, "w0ar": W0ar, "ident4": ident4,
        })
    resT = _run(ncT, t_maps, trace=_trace)
    fs0_rows_u16 = np.concatenate(
        [np.asarray(r["fs0bT"]).view(np.uint16).T for r in resT.results])
    el0_rows = np.concatenate(
        [np.asarray(r["el0i"]).reshape(P, T0_CHUNKS, H).transpose(1, 0, 2)
         .reshape(T0_ROWS, H) for r in resT.results])
    er0_rows = np.concatenate(
        [np.asarray(r["er0i"]).reshape(P, T1_CHUNKS, H).transpose(1, 0, 2)
         .reshape(T1_ROWS, H) for r in resT.results])

    # ---- launch A ----
    eo0, ec0, ech0, el0l = _edge_place(dst0, rank0, start0)
    sfs0, sel0 = _build_streams(fs0_rows_u16, el0_rows, src0,
                                eo0, ec0, ech0, el0l, Khat0, start0, NBLK0)
    er0s = _per_slot_table(er0_rows[order0], NBLK0)
    a_maps = []
    for c in range(NCORES):
        a_maps.append({
            "sfs": sfs0[c].view(NPBF), "sel": sel0[c], "ers": er0s[c],
            "identb": ident_b, "w1b": W1full_b,
        })
    resA = _run(ncA, a_maps, trace=_trace)
    nslot0 = NBLK0 * NCORES * P
    fs1_by_rank = np.empty((nslot0, HD), np.uint16)
    f8_by_rank = np.empty((nslot0, 8), np.float32)
    rr = (np.arange(NBLK0)[:, None, None] * NCORES * P
          + np.arange(P)[None, None, :])
    for c in range(NCORES):
        ranks = (rr + c * P).reshape(-1)
        ob = np.asarray(resA.results[c]["obfs"]).view(np.uint16)
        fs1_by_rank[ranks] = ob.reshape(P, NBLK0, HD).transpose(
            1, 0, 2).reshape(-1, HD)
        o8 = np.asarray(resA.results[c]["obf8"])
        f8_by_rank[ranks] = o8.reshape(P, NBLK0, 8).transpose(
            1, 0, 2).reshape(-1, 8)

    # ---- launch B ----
    eo1, ec1, ech1, el1l = _edge_place(dst1, rank1, start1)
    sfs1, sel1 = _build_streams(
        fs1_by_rank, f8_by_rank[:, 0:4], rank0[src1],
        eo1, ec1, ech1, el1l, Khat1, start1, NBLK1)
    er1_for_slot = f8_by_rank[rank0[map12[order1]]][:, 4:8]
    er1s = _per_slot_table(er1_for_slot, NBLK1)
    b_maps = []
    for c in range(NCORES):
        b_maps.append({
            "sfs": sfs1[c].view(NPBF), "sel": sel1[c], "ers": er1s[c],
            "identb": ident_b,
        })
    resB = _run(ncB, b_maps, trace=_trace)
    nslot1 = NBLK1 * NCORES * P
    log_by_rank = np.empty((nslot1, D), np.float32)
    rr1 = (np.arange(NBLK1)[:, None, None] * NCORES * P
           + np.arange(P)[None, None, :])
    for c in range(NCORES):
        ranks = (rr1 + c * P).reshape(-1)
        ol = np.asarray(resB.results[c]["olog"])
        log_by_rank[ranks] = ol.reshape(P, NBLK1, D).transpose(
            1, 0, 2).reshape(-1, D)
    logits = log_by_rank[rank1[np.arange(N2)]]

    if _collect_times is not None:
        _collect_times.extend([resT, resA, resB])
    return logits.astype(np.float32)


# revision 6
# speedup vs baseline: 1.0639x; 1.0639x over previous
"""Trainium2 Bass kernel for nn_GATSampling (2-layer bipartite GAT, 8 NeuronCores).

Strategy (SPMD over 8 cores, host re-shards between launches):

  Slot-major edge layout: destination nodes are ranked by degree and dealt
  into blocks of 128 consecutive ranks; global block gb -> core gb%8,
  program slot gb//8.  Partition lane p holds the edges of slot p, padded
  to a per-PAIR uniform K.  The segment-sum is a PSUM accumulation of
  matmuls against a constant identity stationary: 512-col quad matmuls
  (4 k-chunks, last dim packed) plus <=3 single-chunk remainders; a DVE
  tensor_reduce folds the quad lanes.

  Launch T: feature transform with column-stationary float32r matmuls
            (psum = W^T @ chunk^T, 512 cols per matmul); el0 computed
            from the bf16 fs output (al-mask stationary), el/er stored
            bf16 to keep the 4-partition output DMAs small.
  Host:     gather per-edge rows into per-core streams laid out
            [lane, h, d, k] per block (k innermost: DVE 2x multiply).
  Launch A: layer-0 edge phase. DMA in 8-block slabs; compute at pair
            granularity: e = el+er, s = exp(prelu(e)), a = s/sum_k s,
            m = fs*a each one instruction per pair; per block quad
            matmuls + fold; ELU via Act relu/exp (pair-batched, -1
            folded into the transpose copy's bias);
            h1ext = elu @ [W1 | W1@al1m | W1@ar1m] in bf16.
  Launch B: layer-2 edge phase; quad-fold and head-mean fused into one
            strided tensor_reduce; outputs logits.
"""
import sys

sys.path.insert(0, "/opt/trn_rl_repo")

import numpy as np
import ml_dtypes

from concourse import bass, mybir, tile, bacc, bass_utils

F32 = mybir.dt.float32
F32R = mybir.dt.float32r
BF16 = mybir.dt.bfloat16
NPBF = ml_dtypes.bfloat16
P = 128
NCORES = 8
NEG_SLOPE = 0.2
H, D = 4, 32
HD = H * D  # 128
PARW = 4

N0, N1, N2 = 200000, 50000, 12500
E0, E1 = 800000, 200000
F_IN = 128

T0_CHUNKS = 196
T0_ROWS = T0_CHUNKS * P
T1_CHUNKS = 49
T1_ROWS = T1_CHUNKS * P

NBLK0 = 50
NBLK1 = 14
PAIRS0 = [(2 * i, 2 * i + 2) for i in range(NBLK0 // 2)]
PAIRS1 = [(2 * i, 2 * i + 2) for i in range(NBLK1 // 2)]
SLABS0 = [(0, 2)] + [(2 + 8 * i, 10 + 8 * i) for i in range(6)]
SLABS1 = [(0, 2), (2, 6), (6, 10), (10, 14)]

EL_PAD = -87.0

_cache = {}


# --------------------------------------------------------------------------
# host-side graph partitioning (index bookkeeping only)
# --------------------------------------------------------------------------
def _partition(dst, n_dst, nblk_core, pairs):
    deg = np.bincount(dst, minlength=n_dst)
    order = np.argsort(-deg, kind="stable")
    rank = np.empty(n_dst, np.int64)
    rank[order] = np.arange(n_dst)
    degs = deg[order]
    Khat = np.empty(nblk_core, np.int64)
    for j in range(nblk_core):
        lo = (j * NCORES) * P
        Khat[j] = degs[lo] if lo < n_dst else 1
    for (j0, j1) in pairs:
        Khat[j0:j1] = max(int(Khat[j0:j1].max()), PARW)
    start = np.zeros(nblk_core + 1, np.int64)
    np.cumsum(Khat, out=start[1:])
    return order, rank, Khat, start


def _edge_place(dst, rank, start):
    r = rank[dst]
    eorder = np.argsort(r, kind="stable")
    rs = r[eorder]
    first = np.searchsorted(rs, rs)
    k = np.arange(len(rs)) - first
    gb = rs >> 7
    core = gb & (NCORES - 1)
    j = gb >> 3
    lane = rs & (P - 1)
    chunk = start[j] + k
    return eorder, core, chunk, lane


def _build_streams(fs_rows_u16, el_rows, src, eorder, core, chunk, lane,
                   Khat, start, nblk_core):
    C = int(start[-1])
    arr_fs = np.zeros((NCORES, C, P, HD), np.uint16)
    arr_el = np.full((NCORES, C, P, H), EL_PAD, np.float32)
    se = src[eorder]
    arr_fs[core, chunk, lane] = fs_rows_u16[se]
    arr_el[core, chunk, lane] = el_rows[se]
    sfs = np.empty((NCORES, P, HD * C), np.uint16)
    sel = np.empty((NCORES, P, H * C), np.float32)
    for j in range(nblk_core):
        s0, K = int(start[j]), int(Khat[j])
        fslab = arr_fs[:, s0:s0 + K].transpose(0, 2, 3, 1)
        sfs[:, :, HD * s0:HD * (s0 + K)] = fslab.reshape(NCORES, P, HD * K)
        eslab = arr_el[:, s0:s0 + K].transpose(0, 2, 3, 1)
        sel[:, :, H * s0:H * (s0 + K)] = eslab.reshape(NCORES, P, H * K)
    return sfs, sel


def _per_slot_table(vals_by_rank, nblk_core):
    nb = nblk_core * NCORES
    v = np.zeros((nb * P, H), np.float32)
    v[:len(vals_by_rank)] = vals_by_rank
    v = v.reshape(nblk_core, NCORES, P, H)
    return np.ascontiguousarray(v.transpose(1, 2, 0, 3)).reshape(
        NCORES, P, nblk_core * H)


# --------------------------------------------------------------------------
# bass programs
# --------------------------------------------------------------------------
def _build_T():
    GC = 4
    ST = 28
    nc = bacc.Bacc("TRN2", target_bir_lowering=False, debug=False)
    f0T = nc.dram_tensor("f0T", [P, T0_CHUNKS * P], F32R, kind="ExternalInput").ap()
    f1T = nc.dram_tensor("f1T", [P, T1_CHUNKS * P], F32R, kind="ExternalInput").ap()
    w0 = nc.dram_tensor("w0", [F_IN, HD], F32R, kind="ExternalInput").ap()
    al0b = nc.dram_tensor("al0b", [HD, H], BF16, kind="ExternalInput").ap()
    w0ar = nc.dram_tensor("w0ar", [F_IN, H], F32R, kind="ExternalInput").ap()
    fs0bT = nc.dram_tensor("fs0bT", [P, T0_ROWS], BF16, kind="ExternalOutput").ap()
    el0T = nc.dram_tensor("el0T", [H, T0_ROWS], BF16, kind="ExternalOutput").ap()
    er0T = nc.dram_tensor("er0T", [H, T1_ROWS], BF16, kind="ExternalOutput").ap()

    with tile.TileContext(nc) as tc:
        with (
            tc.tile_pool(name="const", bufs=1) as cpool,
            tc.tile_pool(name="load", bufs=2) as lpool,
            tc.tile_pool(name="fsout", bufs=2) as fpool,
            tc.tile_pool(name="elout", bufs=2) as epool,
            tc.tile_pool(name="erout", bufs=1) as rpool,
            tc.tile_pool(name="ps", bufs=2, space="PSUM") as ppool,
            tc.tile_pool(name="pse", bufs=2, space="PSUM") as ppool2,
        ):
            w0_sb = cpool.tile([F_IN, HD], F32R)
            nc.sync.dma_start(w0_sb[:], w0)
            al0b_sb = cpool.tile([HD, H], BF16)
            nc.sync.dma_start(al0b_sb[:], al0b)
            w0ar_sb = cpool.tile([F_IN, H], F32R)
            nc.sync.dma_start(w0ar_sb[:], w0ar)

            # ---- feat0 pass: fs (f32r) then el from bf16 fs ----
            for st in range(T0_CHUNKS // ST):
                ld = lpool.tile([P, ST * P], F32R, tag="ld")
                nc.sync.dma_start(ld[:], f0T[:, st * ST * P:(st + 1) * ST * P])
                fst = fpool.tile([P, ST * P], BF16, tag="fst")
                elt = epool.tile([H, ST * P], BF16, tag="elt")
                for g in range(ST // GC):
                    sl = slice(g * GC * P, (g + 1) * GC * P)
                    ps = ppool.tile([P, GC * P], F32, space="PSUM", tag="ps")
                    nc.tensor.matmul(ps[:], lhsT=w0_sb[:], rhs=ld[:, sl],
                                     start=True, stop=True)
                    nc.scalar.copy(fst[:, sl], ps[:])
                    pse = ppool2.tile([H, GC * P], F32, space="PSUM", tag="pse")
                    nc.tensor.matmul(pse[:], lhsT=al0b_sb[:], rhs=fst[:, sl],
                                     start=True, stop=True)
                    if g % 2 == 0:
                        nc.vector.tensor_scalar(
                            out=elt[:, sl], in0=pse[:], scalar1=1.0,
                            scalar2=None, op0=mybir.AluOpType.mult)
                    else:
                        nc.scalar.copy(elt[:, sl], pse[:])
                nc.sync.dma_start(
                    fs0bT[:, st * ST * P:(st + 1) * ST * P], fst[:])
                nc.sync.dma_start(
                    el0T[:, st * ST * P:(st + 1) * ST * P], elt[:])

            # ---- feat1 pass: er direct f32r ----
            ert = rpool.tile([H, T1_ROWS], BF16)
            for st in range(2):
                c0 = st * ST
                c1 = min(T1_CHUNKS, (st + 1) * ST)
                ld = lpool.tile([P, ST * P], F32R, tag="ld")
                nc.sync.dma_start(ld[:, 0:(c1 - c0) * P],
                                  f1T[:, c0 * P:c1 * P])
                for g in range((c1 - c0 + GC - 1) // GC):
                    g0 = g * GC
                    g1 = min(c1 - c0, g0 + GC)
                    pse = ppool2.tile([H, GC * P], F32, space="PSUM", tag="pse")
                    nc.tensor.matmul(
                        pse[:, 0:(g1 - g0) * P], lhsT=w0ar_sb[:],
                        rhs=ld[:, g0 * P:g1 * P], start=True, stop=True)
                    nc.vector.tensor_scalar(
                        out=ert[:, (c0 + g0) * P:(c0 + g1) * P],
                        in0=pse[:, 0:(g1 - g0) * P],
                        scalar1=1.0, scalar2=None, op0=mybir.AluOpType.mult)
            nc.sync.dma_start(er0T, ert[:])

    nc.compile()
    return nc


def _pair_attention(nc, Gv5, Ev, er_b, wpool, K, recip_scale):
    """Pair-fused attention: in-place m = fs*a on Gv5 [P,2,H,D,K]."""
    et = wpool.tile([P, 2, H, K], F32, tag="et")
    nc.vector.tensor_tensor(out=et[:], in0=Ev, in1=er_b,
                            op=mybir.AluOpType.add)
    lr = wpool.tile([P, 2, H, K], F32, tag="lr")
    nc.scalar.activation(out=lr[:], in_=et[:],
                         func=mybir.ActivationFunctionType.Prelu,
                         alpha=NEG_SLOPE)
    s = wpool.tile([P, 2, H, K], BF16, tag="s")
    nc.scalar.activation(out=s[:], in_=lr[:],
                         func=mybir.ActivationFunctionType.Exp)
    ssum = wpool.tile([P, 2, H], F32, tag="ssum")
    nc.vector.tensor_reduce(out=ssum[:], in_=s[:],
                            axis=mybir.AxisListType.X,
                            op=mybir.AluOpType.add)
    if recip_scale != 1.0:
        nc.vector.tensor_scalar(out=ssum[:], in0=ssum[:],
                                scalar1=recip_scale, scalar2=None,
                                op0=mybir.AluOpType.mult)
    rec = wpool.tile([P, 2, H], F32, tag="rec")
    nc.vector.reciprocal(rec[:], ssum[:])
    a = wpool.tile([P, 2, H, K], BF16, tag="a")
    nc.vector.tensor_tensor(
        out=a[:], in0=s[:],
        in1=rec[:].unsqueeze(3).to_broadcast([P, 2, H, K]),
        op=mybir.AluOpType.mult)
    nc.vector.tensor_tensor(
        out=Gv5, in0=Gv5,
        in1=a[:].unsqueeze(3).to_broadcast([P, 2, H, D, K]),
        op=mybir.AluOpType.mult)


def _seg_matmuls(nc, psq, Gvj, identb_sb, K):
    nq = K // PARW
    rem = K - nq * PARW
    Gq = Gvj[:, :, :, 0:nq * PARW].rearrange(
        "p h d (kk par) -> p h d kk par", par=PARW)
    for kk in range(nq):
        nc.tensor.matmul(psq[:], lhsT=identb_sb[:],
                         rhs=Gq[:, :, :, kk, :],
                         start=(kk == 0), stop=(kk == nq - 1 and rem == 0))
    for i in range(rem):
        nc.tensor.matmul(psq[:, :, i], lhsT=identb_sb[:],
                         rhs=Gvj[:, :, :, nq * PARW + i],
                         start=False, stop=(i == rem - 1))


def _build_A(Khat):
    nblk = len(Khat)
    start = np.zeros(nblk + 1, np.int64)
    np.cumsum(Khat, out=start[1:])
    C = int(start[-1])
    maxw = max(int(start[j1] - start[j0]) for j0, j1 in SLABS0)

    nc = bacc.Bacc("TRN2", target_bir_lowering=False, debug=False)
    sfs = nc.dram_tensor("sfs", [P, HD * C], BF16, kind="ExternalInput").ap()
    sel = nc.dram_tensor("sel", [P, H * C], F32, kind="ExternalInput").ap()
    ers = nc.dram_tensor("ers", [P, nblk * H], F32, kind="ExternalInput").ap()
    identb = nc.dram_tensor("identb", [P, P], BF16, kind="ExternalInput").ap()
    w1b = nc.dram_tensor("w1b", [HD, 136], BF16, kind="ExternalInput").ap()
    obfs = nc.dram_tensor("obfs", [P, nblk * HD], BF16, kind="ExternalOutput").ap()
    obf8 = nc.dram_tensor("obf8", [P, nblk * 8], F32, kind="ExternalOutput").ap()

    with tile.TileContext(nc) as tc:
        with (
            tc.tile_pool(name="const", bufs=1) as cpool,
            tc.tile_pool(name="gload", bufs=2) as gpool,
            tc.tile_pool(name="eload", bufs=2) as epool,
            tc.tile_pool(name="work", bufs=3) as wpool,
            tc.tile_pool(name="pwork", bufs=3) as qpool,
            tc.tile_pool(name="stage", bufs=1) as spool,
            tc.tile_pool(name="psm", bufs=2, space="PSUM") as ppool,
            tc.tile_pool(name="pst", bufs=2, space="PSUM") as ppool2,
            tc.tile_pool(name="ps3", bufs=2, space="PSUM") as ppool3,
        ):
            identb_sb = cpool.tile([P, P], BF16)
            nc.sync.dma_start(identb_sb[:], identb)
            w1b_sb = cpool.tile([HD, 136], BF16)
            nc.sync.dma_start(w1b_sb[:], w1b)
            ers_sb = cpool.tile([P, nblk * H], F32)
            nc.sync.dma_start(ers_sb[:], ers)
            ofs_st = spool.tile([P, nblk * HD], BF16)
            of8_st = spool.tile([P, nblk * 8], F32)

            slab_tiles = {}
            for (sj0, sj1) in SLABS0:
                s0 = int(start[sj0])
                w = int(start[sj1]) - s0
                Gg = gpool.tile([P, HD * maxw], BF16, tag="G")
                nc.sync.dma_start(Gg[:, 0:HD * w],
                                  sfs[:, HD * s0:HD * (s0 + w)])
                Eg = epool.tile([P, H * maxw], F32, tag="E")
                nc.sync.dma_start(Eg[:, 0:H * w],
                                  sel[:, H * s0:H * (s0 + w)])
                for j in range(sj0, sj1):
                    slab_tiles[j] = (Gg, Eg, s0)

                for (j0, j1) in [p for p in PAIRS0 if sj0 <= p[0] < sj1]:
                    K = int(Khat[j0])
                    off = int(start[j0]) - s0
                    Gv5 = Gg[:, HD * off:HD * (off + 2 * K)].rearrange(
                        "p (j h d k) -> p j h d k", j=2, h=H, d=D)
                    Ev = Eg[:, H * off:H * (off + 2 * K)].rearrange(
                        "p (j h k) -> p j h k", j=2, h=H)
                    er_b = ers_sb[:, j0 * H:j1 * H].rearrange(
                        "p (j h) -> p j h", j=2).unsqueeze(3).to_broadcast(
                        [P, 2, H, K])
                    _pair_attention(nc, Gv5, Ev, er_b, wpool, K, 1.0)
                    yp = qpool.tile([P, 2, HD], F32, tag="yp")
                    ps3p = ppool3.tile([P, 2, 136], F32, space="PSUM",
                                       tag="ps3")
                    for jj in range(2):
                        psq = ppool.tile([P, HD, PARW], F32, space="PSUM",
                                         tag="psq")
                        _seg_matmuls(nc, psq, Gv5[:, jj], identb_sb, K)
                        nc.vector.tensor_reduce(out=yp[:, jj, :], in_=psq[:],
                                                axis=mybir.AxisListType.X,
                                                op=mybir.AluOpType.add)
                    r1p = qpool.tile([P, 2, HD], BF16, tag="r1p")
                    nc.scalar.activation(out=r1p[:], in_=yp[:],
                                         func=mybir.ActivationFunctionType.Relu)
                    ngp = qpool.tile([P, 2, HD], F32, tag="ngp")
                    nc.scalar.activation(out=ngp[:], in_=yp[:],
                                         func=mybir.ActivationFunctionType.Relu,
                                         scale=-1.0)
                    exp2 = qpool.tile([P, 2, HD], BF16, tag="exp2")
                    nc.scalar.activation(out=exp2[:], in_=ngp[:],
                                         func=mybir.ActivationFunctionType.Exp,
                                         scale=-1.0)
                    elu1p = qpool.tile([P, 2, HD], BF16, tag="elu1p")
                    nc.vector.tensor_tensor(out=elu1p[:], in0=r1p[:],
                                            in1=exp2[:],
                                            op=mybir.AluOpType.add)
                    for jj in range(2):
                        pst = ppool2.tile([P, P], BF16, space="PSUM",
                                          tag="pst")
                        nc.tensor.transpose(out=pst[:], in_=elu1p[:, jj, :],
                                            identity=identb_sb[:])
                        eluT = wpool.tile([P, P], BF16, tag="eluT")
                        nc.scalar.activation(
                            out=eluT[:], in_=pst[:],
                            func=mybir.ActivationFunctionType.Copy,
                            bias=-1.0)
                        nc.tensor.matmul(ps3p[:, jj, :], lhsT=eluT[:],
                                         rhs=w1b_sb[:], start=True, stop=True)
                    nc.scalar.copy(ofs_st[:, j0 * HD:(j0 + 2) * HD],
                                   ps3p[:, :, 0:HD])
                    nc.scalar.copy(of8_st[:, j0 * 8:(j0 + 2) * 8],
                                   ps3p[:, :, HD:HD + 8])
            nc.sync.dma_start(obfs, ofs_st[:])
            nc.sync.dma_start(obf8, of8_st[:])

    nc.compile()
    return nc


def _build_B(Khat):
    nblk = len(Khat)
    start = np.zeros(nblk + 1, np.int64)
    np.cumsum(Khat, out=start[1:])
    C = int(start[-1])
    maxw = max(int(start[j1] - start[j0]) for j0, j1 in SLABS1)

    nc = bacc.Bacc("TRN2", target_bir_lowering=False, debug=False)
    sfs = nc.dram_tensor("sfs", [P, HD * C], BF16, kind="ExternalInput").ap()
    sel = nc.dram_tensor("sel", [P, H * C], F32, kind="ExternalInput").ap()
    ers = nc.dram_tensor("ers", [P, nblk * H], F32, kind="ExternalInput").ap()
    identb = nc.dram_tensor("identb", [P, P], BF16, kind="ExternalInput").ap()
    olog = nc.dram_tensor("olog", [P, nblk * D], F32, kind="ExternalOutput").ap()

    with tile.TileContext(nc) as tc:
        with (
            tc.tile_pool(name="const", bufs=1) as cpool,
            tc.tile_pool(name="gload", bufs=2) as gpool,
            tc.tile_pool(name="eload", bufs=2) as epool,
            tc.tile_pool(name="work", bufs=3) as wpool,
            tc.tile_pool(name="stage", bufs=1) as spool,
            tc.tile_pool(name="psm", bufs=3, space="PSUM") as ppool,
        ):
            identb_sb = cpool.tile([P, P], BF16)
            nc.sync.dma_start(identb_sb[:], identb)
            ers_sb = cpool.tile([P, nblk * H], F32)
            nc.sync.dma_start(ers_sb[:], ers)
            olog_st = spool.tile([P, nblk * D], F32)

            for (sj0, sj1) in SLABS1:
                s0 = int(start[sj0])
                w = int(start[sj1]) - s0
                Gg = gpool.tile([P, HD * maxw], BF16, tag="G")
                nc.sync.dma_start(Gg[:, 0:HD * w],
                                  sfs[:, HD * s0:HD * (s0 + w)])
                Eg = epool.tile([P, H * maxw], F32, tag="E")
                nc.sync.dma_start(Eg[:, 0:H * w],
                                  sel[:, H * s0:H * (s0 + w)])
                for (j0, j1) in [p for p in PAIRS1 if sj0 <= p[0] < sj1]:
                    K = int(Khat[j0])
                    off = int(start[j0]) - s0
                    Gv5 = Gg[:, HD * off:HD * (off + 2 * K)].rearrange(
                        "p (j h d k) -> p j h d k", j=2, h=H, d=D)
                    Ev = Eg[:, H * off:H * (off + 2 * K)].rearrange(
                        "p (j h k) -> p j h k", j=2, h=H)
                    er_b = ers_sb[:, j0 * H:j1 * H].rearrange(
                        "p (j h) -> p j h", j=2).unsqueeze(3).to_broadcast(
                        [P, 2, H, K])
                    _pair_attention(nc, Gv5, Ev, er_b, wpool, K, 4.0)
                    for jj in range(2):
                        j = j0 + jj
                        psq = ppool.tile([P, HD, PARW], F32, space="PSUM",
                                         tag="psq")
                        _seg_matmuls(nc, psq, Gv5[:, jj], identb_sb, K)
                        nc.vector.tensor_reduce(
                            out=olog_st[:, j * D:(j + 1) * D],
                            in_=psq[:].rearrange(
                                "p (h d) par -> p d h par", h=H),
                            axis=mybir.AxisListType.XY,
                            op=mybir.AluOpType.add)
            nc.sync.dma_start(olog, olog_st[:])

    nc.compile()
    return nc


def _get_programs(Khat0, Khat1):
    key = (tuple(Khat0), tuple(Khat1))
    if key not in _cache:
        _cache[key] = (_build_T(), _build_A(Khat0), _build_B(Khat1))
    return _cache[key]


def _run(nc, in_maps, trace=False):
    return bass_utils.run_bass_kernel_spmd(
        nc, in_maps, list(range(NCORES)), trace=trace)


# --------------------------------------------------------------------------
# main entry
# --------------------------------------------------------------------------
def kernel(feat0, feat1, src0, dst0, src1, dst1, map12,
           W0, al0, ar0, W1, al1, ar1, _collect_times=None, _trace=False):
    feat0 = np.asarray(feat0, np.float32)
    feat1 = np.asarray(feat1, np.float32)
    src0 = np.asarray(src0).astype(np.int64)
    dst0 = np.asarray(dst0).astype(np.int64)
    src1 = np.asarray(src1).astype(np.int64)
    dst1 = np.asarray(dst1).astype(np.int64)
    map12 = np.asarray(map12).astype(np.int64)
    W0 = np.asarray(W0, np.float32)
    W1 = np.asarray(W1, np.float32)
    al0 = np.asarray(al0, np.float32); ar0 = np.asarray(ar0, np.float32)
    al1 = np.asarray(al1, np.float32); ar1 = np.asarray(ar1, np.float32)

    al0m = np.zeros((HD, H), np.float32)
    ar0m = np.zeros((HD, H), np.float32)
    al1m = np.zeros((HD, H), np.float32)
    ar1m = np.zeros((HD, H), np.float32)
    for h in range(H):
        al0m[h * D:(h + 1) * D, h] = al0[h]
        ar0m[h * D:(h + 1) * D, h] = ar0[h]
        al1m[h * D:(h + 1) * D, h] = al1[h]
        ar1m[h * D:(h + 1) * D, h] = ar1[h]
    W0ar = (W0 @ ar0m).astype(np.float32)
    W1full_b = np.concatenate(
        [W1, W1 @ al1m, W1 @ ar1m], axis=1).astype(NPBF)
    ident_b = np.eye(P, dtype=NPBF)

    order0, rank0, Khat0, start0 = _partition(dst0, N1, NBLK0, PAIRS0)
    order1, rank1, Khat1, start1 = _partition(dst1, N2, NBLK1, PAIRS1)

    ncT, ncA, ncB = _get_programs(Khat0, Khat1)

    # ---- launch T ----
    f0pad = np.zeros((NCORES * T0_ROWS, F_IN), np.float32)
    f0pad[:N0] = feat0
    f1pad = np.zeros((NCORES * T1_ROWS, F_IN), np.float32)
    f1pad[:N1] = feat1
    t_maps = []
    for c in range(NCORES):
        t_maps.append({
            "f0T": np.ascontiguousarray(
                f0pad[c * T0_ROWS:(c + 1) * T0_ROWS].T),
            "f1T": np.ascontiguousarray(
                f1pad[c * T1_ROWS:(c + 1) * T1_ROWS].T),
            "w0": W0, "al0b": al0m.astype(NPBF), "w0ar": W0ar,
        })
    resT = _run(ncT, t_maps, trace=_trace)
    fs0_rows_u16 = np.concatenate(
        [np.asarray(r["fs0bT"]).view(np.uint16).T for r in resT.results])
    el0_rows = np.concatenate(
        [np.asarray(r["el0T"]).astype(np.float32).T for r in resT.results])
    er0_rows = np.concatenate(
        [np.asarray(r["er0T"]).astype(np.float32).T for r in resT.results])

    # ---- launch A ----
    eo0, ec0, ech0, el0l = _edge_place(dst0, rank0, start0)
    sfs0, sel0 = _build_streams(fs0_rows_u16, el0_rows, src0,
                                eo0, ec0, ech0, el0l, Khat0, start0, NBLK0)
    er0s = _per_slot_table(er0_rows[order0], NBLK0)
    a_maps = []
    for c in range(NCORES):
        a_maps.append({
            "sfs": sfs0[c].view(NPBF), "sel": sel0[c], "ers": er0s[c],
            "identb": ident_b, "w1b": W1full_b,
        })
    resA = _run(ncA, a_maps, trace=_trace)
    nslot0 = NBLK0 * NCORES * P
    fs1_by_rank = np.empty((nslot0, HD), np.uint16)
    f8_by_rank = np.empty((nslot0, 8), np.float32)
    rr = (np.arange(NBLK0)[:, None, None] * NCORES * P
          + np.arange(P)[None, None, :])
    for c in range(NCORES):
        ranks = (rr + c * P).reshape(-1)
        ob = np.asarray(resA.results[c]["obfs"]).view(np.uint16)
        fs1_by_rank[ranks] = ob.reshape(P, NBLK0, HD).transpose(
            1, 0, 2).reshape(-1, HD)
        o8 = np.asarray(resA.results[c]["obf8"])
        f8_by_rank[ranks] = o8.reshape(P, NBLK0, 8).transpose(
            1, 0, 2).reshape(-1, 8)

    # ---- launch B ----
    eo1, ec1, ech1, el1l = _edge_place(dst1, rank1, start1)
    sfs1, sel1 = _build_streams(
        fs1_by_rank, f8_by_rank[:, 0:4], rank0[src1],
        eo1, ec1, ech1, el1l, Khat1, start1, NBLK1)
    er1_for_slot = f8_by_rank[rank0[map12[order1]]][:, 4:8]
    er1s = _per_slot_table(er1_for_slot, NBLK1)
    b_maps = []
    for c in range(NCORES):
        b_maps.append({
            "sfs": sfs1[c].view(NPBF), "sel": sel1[c], "ers": er1s[c],
            "identb": ident_b,
        })
    resB = _run(ncB, b_maps, trace=_trace)
    nslot1 = NBLK1 * NCORES * P
    log_by_rank = np.empty((nslot1, D), np.float32)
    rr1 = (np.arange(NBLK1)[:, None, None] * NCORES * P
           + np.arange(P)[None, None, :])
    for c in range(NCORES):
        ranks = (rr1 + c * P).reshape(-1)
        ol = np.asarray(resB.results[c]["olog"])
        log_by_rank[ranks] = ol.reshape(P, NBLK1, D).transpose(
            1, 0, 2).reshape(-1, D)
    logits = log_by_rank[rank1[np.arange(N2)]]

    if _collect_times is not None:
        _collect_times.extend([resT, resA, resB])
    return logits.astype(np.float32)
